# revision 10
# baseline (speedup 1.0000x reference)
"""Trainium2 Bass kernel for DeformableMultiHeadedAttention.

Data-parallel over batch B=8 across 8 NeuronCores (one batch element per
core, identical programs, no collectives).

Per-core pipeline (heavy matmuls fp8e4 DoubleRow with f32 accumulate):
  1. q,k,v [4096,512] f32 -> SWDGE cast-DMA -> DRAM fp8e4 -> HWDGE
     DMA-transpose of the uint16 (fp8-pair) view -> feature-pair-major
     XT8 [128, 2, tok] chunks in SBUF. The pair axis is exactly the
     DoubleRow contraction pair.
  2. Projections on PE in fp8 DoubleRow (weights host-packed
     [p, g, j, of] = W[g*256+2p+j, of]*32, eviction rescales by 1/32):
     K'T/Q'T feature-major; V' token-major with bias via K=1 rank-1 matmul.
  3. Q pooling (AvgPool k=5, stride 1, zero pad) as 3 shifted adds (DVE);
     the 1/5 is folded into the softmax exp scale.
  4. DSA (windows of 8 tokens): per 128-token tile, 8 heads: S_T[k,q] on PE
     bf16 (heads bank-segregated), exp on ACT, block-diag mask mul on DVE,
     attn@V plus N=1 denominator matmul sharing the lhsT, per-partition
     1/den scale on DVE. Token-major DSA output -> DRAM (bf16).
  5. DRAM round-trips: DMA-transpose -> attn_xT feature-major; strided
     SWDGE cast-gather -> PV window-major fp8e4 [kw, (cc, slot, head, hd)].
  6. win_tok LayerNorm (moments via ones-matmuls) + exact GELU, pq/pk
     projections, PSA over 512 windows: es in fp8e4, denominator via
     DoubleRow matmul against a const-(1/16) stationary (so recip = 16/sum),
     attn8 = es*recip16 in fp8e4 plane-pair layout.
  7. pout = pattn @ PV in fp8 DoubleRow per (head, slot); Z = pout/16 +
     attn_x via one DVE scalar_tensor_tensor; final out = Z @ Wo + bo with
     Z as the stationary operand -> token-major bf16 output (host casts to
     f32).
"""

import sys
from contextlib import ExitStack

for _p in ("/opt/trn_rl_repo/concourse", "/opt/trn_rl_repo"):
    if _p not in sys.path:
        sys.path.insert(0, _p)

import numpy as np
import ml_dtypes

import concourse.bass as bass
import concourse.mybir as mybir
import concourse.tile as tile
from concourse import bacc
from concourse.tile import add_dep_helper
from concourse.bass_utils import run_bass_kernel_spmd

BF16 = mybir.dt.bfloat16
F32 = mybir.dt.float32
FP8 = mybir.dt.float8e4
U16 = mybir.dt.uint16
AF = mybir.ActivationFunctionType
ALU = mybir.AluOpType
PM = mybir.MatmulPerfMode

B, M, D = 8, 4096, 512
H, HD = 8, 64
WIN = 7
PW = WIN + 1
QNB = 5
QLEN = 3584
WN = M // PW
SCALE = D ** -0.5
EPS = 1e-5
NCHUNK = 8
CH = 512
WSC = 32.0          # host-side premultiplier on fp8 projection weights
ASC = 16.0          # attn8 = es * (ASC/den); pout rescaled by 1/ASC
PERM = [(h % 2) * 4 + h // 2 for h in range(H)]  # head -> DSA psum slot


def build_program():
    nc = bacc.Bacc("TRN2", target_bir_lowering=False, debug=False, num_devices=8)

    t = {}
    t["q_in"] = nc.dram_tensor("q", [M, D], F32, kind="ExternalInput")
    t["k_in"] = nc.dram_tensor("k", [M, D], F32, kind="ExternalInput")
    t["v_in"] = nc.dram_tensor("v", [M, D], F32, kind="ExternalInput")
    for nm in ("wq8", "wk8"):
        t[nm] = nc.dram_tensor(nm, [128, 2, 2, D], FP8, kind="ExternalInput")
    for nm in ("wv", "wpq", "wpk", "wo"):
        t[nm] = nc.dram_tensor(nm, [D, D], BF16, kind="ExternalInput")
    for nm in ("bq_c", "bk_c", "bpq_c", "bpk_c", "ln_g_c", "ln_b_c"):
        t[nm] = nc.dram_tensor(nm, [128, 4], F32, kind="ExternalInput")
    t["bv_r"] = nc.dram_tensor("bv_r", [1, D], BF16, kind="ExternalInput")
    t["bo_r"] = nc.dram_tensor("bo_r", [1, D], BF16, kind="ExternalInput")
    t["bmask"] = nc.dram_tensor("bmask", [128, 128], BF16, kind="ExternalInput")
    t["out"] = nc.dram_tensor("out", [QLEN, D], BF16, kind="ExternalOutput")
    t["axd"] = nc.dram_tensor("axd_s", [M, D], BF16, kind="Internal")
    t["q8"] = nc.dram_tensor("q8_s", [M, D], FP8, kind="Internal")
    t["k8"] = nc.dram_tensor("k8_s", [M, D], FP8, kind="Internal")
    t["vb"] = nc.dram_tensor("vb_s", [M, D], BF16, kind="Internal")

    with tile.TileContext(nc) as tc:
        _build(nc, tc, t)
    nc.compile()
    return nc


def _build(nc, tc, t):
    axd, out = t["axd"], t["out"]
    # uint16 (fp8-pair) views of the casted inputs: [chunk, g, tok, pair-col]
    xu = {
        nm: t[nm].ap().bitcast(U16).rearrange(
            "(c t) (g f) -> c g t f", t=CH, f=128)
        for nm in ("q8", "k8")
    }

    with ExitStack() as octx:
        singles = octx.enter_context(tc.tile_pool(name="singles", bufs=1))

        W = {}
        for nm in ("wq8", "wk8"):
            W[nm] = singles.tile([128, 2, 2, D], FP8, tag=nm, name=f"w_{nm}")
            nc.scalar.dma_start(out=W[nm][:], in_=t[nm][:, :, :, :])
        W["wv"] = singles.tile([128, 4, D], BF16, tag="wv", name="w_wv")
        nc.scalar.dma_start(out=W["wv"][:],
                            in_=t["wv"].ap().rearrange("(c p) d -> p c d", p=128))
        bias_cols = {}
        for nm in ("bq_c", "bk_c"):
            bias_cols[nm] = singles.tile([128, 4], F32, tag=nm, name=f"bc_{nm}")
            nc.scalar.dma_start(out=bias_cols[nm][:], in_=t[nm][:, :])
        bv_sb = singles.tile([1, D], BF16)
        nc.scalar.dma_start(out=bv_sb[:], in_=t["bv_r"][:, :])
        mask_sb = singles.tile([128, 128], BF16)
        nc.scalar.dma_start(out=mask_sb[:], in_=t["bmask"][:, :])
        ones_row = singles.tile([1, 128], BF16)
        nc.vector.memset(ones_row[:], 1.0)
        ones_full = singles.tile([128, 128], BF16)
        nc.vector.memset(ones_full[:], 1.0)
        inv16_full = singles.tile([128, 2, 128], FP8)
        nc.vector.memset(inv16_full[:], 1.0 / ASC)
        eps_sb = singles.tile([128, 1], F32)
        nc.vector.memset(eps_sb[:], EPS)

        axd_writers = []
        p2a = octx.enter_context(tc.tile_pool(name="p2a", bufs=1))
        axt = p2a.tile([128, 4, M], BF16, tag="axt")

        # ================= phase 2 weights (early load) =================
        for nm in ("wpq", "wpk", "wo"):
            W[nm] = singles.tile([128, 4, D], BF16, tag=nm, name=f"w_{nm}")
            nc.scalar.dma_start(out=W[nm][:],
                                in_=t[nm].ap().rearrange("(c p) d -> p c d", p=128))
        for nm in ("bpq_c", "bpk_c", "ln_g_c", "ln_b_c"):
            bias_cols[nm] = singles.tile([128, 4], F32, tag=nm, name=f"bc_{nm}")
            nc.scalar.dma_start(out=bias_cols[nm][:], in_=t[nm][:, :])
        bo_sb = singles.tile([1, D], BF16)
        nc.scalar.dma_start(out=bo_sb[:], in_=t["bo_r"][:, :])

        # ================= phase 1 =================
        with ExitStack() as ctx:
            p1 = ctx.enter_context(tc.tile_pool(name="p1", bufs=1))
            kT = p1.tile([128, 4, 3, CH], BF16, tag="kT")        # ring of 3 chunks
            qpT = p1.tile([128, 4, 3, CH], BF16, tag="qpT")      # ring of 3 chunks
            vtm = p1.tile([128, 12, 8, 65], BF16, tag="vtm")     # ring of 12 tiles
            nc.vector.memset(vtm[:, :, :, 64:65], 1.0)           # ones col for denoms
            qraw = p1.tile([128, 4, M + 4], BF16, tag="qraw")    # full, padded +-2
            nc.vector.memset(qraw[:, :, 0:2], 0.0)
            nc.vector.memset(qraw[:, :, M + 2:M + 4], 0.0)

            xtp = ctx.enter_context(tc.tile_pool(name="xtp", bufs=2))
            ps_proj = ctx.enter_context(tc.tile_pool(name="ps_proj", bufs=2, space="PSUM"))
            ps_st = ctx.enter_context(tc.tile_pool(name="ps_st", bufs=2, space="PSUM"))
            ps_out = ctx.enter_context(tc.tile_pool(name="ps_out", bufs=1, space="PSUM"))
            dsa_sb = ctx.enter_context(tc.tile_pool(name="dsa_sb", bufs=2))
            pool_tmp = ctx.enter_context(tc.tile_pool(name="pool_tmp", bufs=2))
            ax_pool = ctx.enter_context(tc.tile_pool(name="ax_sb", bufs=2))

            cast_insts = {"q8": [], "k8": [], "vb": []}

            def issue_casts(lo, hi):
                for nm, srcd in (("q8", t["q_in"]), ("k8", t["k_in"]),
                                 ("vb", t["v_in"])):
                    ci = nc.gpsimd.dma_start(
                        out=t[nm][lo * CH:hi * CH, :],
                        in_=srcd[lo * CH:hi * CH, :])
                    cast_insts[nm].append(((lo, hi), ci))

            for lo, hi in ((0, 1), (1, 2), (2, 4), (4, 6), (6, 8)):
                issue_casts(lo, hi)

            def load_xt(nm, c):
                """Transpose the fp8-pair (u16) image of chunk c -> SBUF."""
                xt = xtp.tile([128, 2, CH], U16, tag=f"xt_{nm}", name=f"xt_{nm}_{c}")
                for g in range(2):
                    ti = nc.sync.dma_start(out=xt[:, g, :], in_=xu[nm][c, g],
                                           transpose=True)
                    for (lo, hi), ci in cast_insts[nm]:
                        if lo <= c < hi:
                            add_dep_helper(ti.ins, ci.ins,
                                           reason="transpose reads cast output")
                # fp8 pair view [128, g, j, tok]
                return xt[:].bitcast(FP8).rearrange("p g (t j) -> p g j t", j=2)

            def proj_fm_group(xp, wname, bname, dst_fn, j):
                ps = ps_proj.tile([128, CH], F32, tag="proj",
                                  name=f"ps_{wname}_{j}")
                for g in range(2):
                    nc.tensor.matmul(ps[:], W[wname][:, g, :, j * 128:(j + 1) * 128],
                                     xp[:, g, :, :], start=(g == 0), stop=(g == 1),
                                     perf_mode=PM.DoubleRow)
                nc.scalar.activation(dst_fn(j), ps[:], AF.Identity,
                                     bias=bias_cols[bname][:, j:j + 1],
                                     scale=1.0 / WSC)

            def load_vxt(c):
                """v keeps the bf16 path: fp8 v or Wv alone costs ~2.6e-2 rel
                err (v' flows straight into attn_x and the output)."""
                vxt = xtp.tile([128, 4, CH], BF16, tag="xt_v", name=f"xt_v_{c}")
                for dc in range(4):
                    ti = nc.sync.dma_start(
                        out=vxt[:, dc, :],
                        in_=t["vb"][c * CH:(c + 1) * CH, dc * 128:(dc + 1) * 128],
                        transpose=True)
                    for (lo, hi), ci in cast_insts["vb"]:
                        if lo <= c < hi:
                            add_dep_helper(ti.ins, ci.ins,
                                           reason="transpose reads cast output")
                return vxt

            def proj_v_group(vxt, c, tt):
                ps = ps_proj.tile([128, D], F32, tag="proj", name=f"ps_v_{tt}")
                for dk in range(4):
                    nc.tensor.matmul(ps[:], vxt[:, dk, tt * 128:(tt + 1) * 128],
                                     W["wv"][:, dk, :], start=(dk == 0), stop=False,
                                     skip_group_check=True)
                nc.tensor.matmul(ps[:], ones_row[:], bv_sb[:], start=False,
                                 stop=True, skip_group_check=True)
                nc.scalar.copy(vtm[:, (c * 4 + tt) % 12, :, 0:64],
                               ps[:].rearrange("p (h d) -> p h d", h=H))

            def pool_chunk(c):
                base = c * CH
                ta = pool_tmp.tile([128, 4, CH + 2], BF16, tag="ta")
                nc.vector.tensor_add(ta[:], qraw[:, :, base:base + CH + 2],
                                     qraw[:, :, base + 1:base + CH + 3])
                tb = pool_tmp.tile([128, 4, CH], BF16, tag="tb")
                nc.vector.tensor_add(tb[:], ta[:, :, 0:CH], ta[:, :, 2:CH + 2])
                nc.vector.tensor_add(qpT[:, :, c % 3, :], tb[:],
                                     qraw[:, :, base + 4:base + CH + 4])

            def dsa_scores(c, lt):
                """MM1 + exp + mask for tile lt of chunk c -> masked sbuf tile."""
                st = ps_st.tile([128, 8, 128], F32, tag="st", name=f"st_{c}_{lt}")
                for h in range(H):
                    hp = PERM[h]
                    base = (h % 2) * 64
                    lhsT = kT[base:base + 64, h // 2, c % 3, lt * 128:(lt + 1) * 128]
                    rhs = qpT[base:base + 64, h // 2, c % 3, lt * 128:(lt + 1) * 128]
                    nc.tensor.matmul(st[:, hp, :], lhsT, rhs, start=True, stop=True,
                                     skip_group_check=True)
                expS = dsa_sb.tile([128, 8, 128], BF16, tag="expS",
                                   name=f"expS_{c}_{lt}")
                nc.scalar.activation(expS[:], st[:], AF.Exp, scale=SCALE / QNB)
                masked = dsa_sb.tile([128, 8, 128], BF16, tag="masked",
                                     name=f"masked_{c}_{lt}")
                nc.vector.tensor_mul(masked[:], expS[:],
                                     mask_sb[:].unsqueeze(1).to_broadcast((128, 8, 128)))
                return masked

            def dsa_out(c, lt, masked, ax_out):
                """attn@V with ones-col denominators, then normalize."""
                outp = ps_out.tile([128, 2, 512], F32, tag="outp",
                                   name=f"outp_{c}_{lt}")
                for h in range(H):
                    hp = PERM[h]
                    nc.tensor.matmul(outp[:, h // 4, (h % 4) * 65:(h % 4) * 65 + 65],
                                     masked[:, hp, :],
                                     vtm[:, (c * 4 + lt) % 12, h, :],
                                     start=True, stop=True, skip_group_check=True)
                recip = dsa_sb.tile([128, 2, 4], F32, tag="recip",
                                    name=f"recip_{c}_{lt}")
                den_view = bass.AP(outp.tensor, outp[:].offset + 64,
                                   [outp[:].ap[0], [512, 2], [65, 4]])
                nc.vector.reciprocal(recip[:], den_view)
                # V' already contains +bv (rank-1 matmul in proj_v); attention
                # weights sum to 1 after the 1/den scale, so bias is exact.
                av_view = bass.AP(outp.tensor, outp[:].offset,
                                  [outp[:].ap[0], [512, 2], [65, 4], [1, 64]])
                nc.vector.tensor_mul(
                    ax_out.rearrange("p (a b d) -> p a b d", a=2, b=4),
                    av_view,
                    recip[:].unsqueeze(3).to_broadcast((128, 2, 4, 64)))

            def dsa_group_list(c, ax):
                masked = {}
                g = []
                g.append(lambda: masked.__setitem__(0, dsa_scores(c, 0)))
                g.append(lambda: masked.__setitem__(1, dsa_scores(c, 1)))
                g.append(lambda: dsa_out(c, 0, masked.pop(0), ax[:, 0, :]))
                g.append(lambda: masked.__setitem__(2, dsa_scores(c, 2)))
                g.append(lambda: dsa_out(c, 1, masked.pop(1), ax[:, 1, :]))
                g.append(lambda: masked.__setitem__(3, dsa_scores(c, 3)))
                g.append(lambda: dsa_out(c, 2, masked.pop(2), ax[:, 2, :]))
                g.append(lambda: dsa_out(c, 3, masked.pop(3), ax[:, 3, :]))
                return g

            def store_ax(c, ax):
                dst = axd.ap().rearrange("(cc lt p) d -> cc p lt d", lt=4, p=128)[c]
                wi = nc.gpsimd.dma_start(out=dst, in_=ax[:])
                axd_writers.append(wi)

            for c in range(NCHUNK + 2):
                pgroups = []
                if c < NCHUNK:
                    qxt = load_xt("q8", c)
                    kxt = load_xt("k8", c)
                    vxt = load_vxt(c)
                    for j in range(4):
                        pgroups.append(lambda j=j, x=qxt, c=c: proj_fm_group(
                            x, "wq8", "bq_c",
                            lambda jj, c=c: qraw[:, jj, 2 + c * CH:2 + (c + 1) * CH], j))
                    for j in range(4):
                        pgroups.append(lambda j=j, x=kxt, c=c: proj_fm_group(
                            x, "wk8", "bk_c", lambda jj, c=c: kT[:, jj, c % 3, :], j))
                    for tt in range(4):
                        pgroups.append(lambda tt=tt, x=vxt, c=c: proj_v_group(x, c, tt))
                dgroups = []
                ax = None
                if c >= 2:
                    ax = ax_pool.tile([128, 4, D], BF16, tag="ax", name=f"ax_{c - 2}")
                    dgroups = dsa_group_list(c - 2, ax)
                # weave: spread D groups evenly through the P stream;
                # pool(c-1) after the 4 Q-projection groups
                npg, ndg = len(pgroups), len(dgroups)
                dpos = {int(round((k + 1) * npg / (ndg + 1))): k for k in range(ndg)} \
                    if npg else {}
                for i in range(max(npg, 1)):
                    if i < npg:
                        pgroups[i]()
                    if i == 3 and 1 <= c <= NCHUNK:
                        pool_chunk(c - 1)
                    if i in dpos:
                        dgroups[dpos[i]]()
                if not pgroups:
                    if 1 <= c <= NCHUNK:
                        pool_chunk(c - 1)
                    for g in dgroups:
                        g()
                if ax is not None:
                    store_ax(c - 2, ax)

        # ================= phase 2 =================
        with ExitStack() as ctx:
            p2 = ctx.enter_context(tc.tile_pool(name="p2", bufs=1))
            ps2 = ctx.enter_context(tc.tile_pool(name="ps2", bufs=3, space="PSUM"))
            ps2b = ctx.enter_context(tc.tile_pool(name="ps2b", bufs=2, space="PSUM"))
            sb2 = ctx.enter_context(tc.tile_pool(name="sb2", bufs=2))

            pv = p2.tile([128, 4, WIN, D], FP8, tag="pv")
            # issue axt transposes + pv cast-gathers in dependency-arrival order
            srcv = axd.ap().rearrange("(cc p w) d -> cc p w d", p=128, w=PW)
            for c in range(NCHUNK):
                for j in range(4):
                    ti = nc.sync.dma_start(
                        out=axt[:, j, c * CH:(c + 1) * CH],
                        in_=axd[c * CH:(c + 1) * CH, j * 128:(j + 1) * 128],
                        transpose=True)
                    add_dep_helper(ti.ins, axd_writers[c].ins,
                                   reason="axt transpose reads axd chunk")
                if c % 2 == 1:
                    cc = c // 2
                    gi = nc.gpsimd.dma_start(out=pv[:, cc, :, :],
                                             in_=srcv[cc, :, 1:PW, :])
                    add_dep_helper(gi.ins, axd_writers[2 * cc].ins, reason="pv gather")
                    add_dep_helper(gi.ins, axd_writers[2 * cc + 1].ins, reason="pv gather")

            # ---- win_tok LN + GELU ----
            wt_view = axt[:, :, 0::PW]  # [128, 4, 512]
            wtn = p2.tile([128, 4, WN], BF16, tag="wtn")
            RN = WN
            for r in range(1):
                wsq = sb2.tile([128, 4, RN], BF16, tag="wsq", bufs=1, name=f"wsq_{r}")
                nc.scalar.activation(wsq[:], wt_view[:, :, r * RN:(r + 1) * RN],
                                     AF.Square)
                ps_mu = ps2.tile([128, RN], F32, tag="ps2", name=f"psmu_{r}")
                ps_var = ps2.tile([128, RN], F32, tag="ps2", name=f"psvar_{r}")
                for j in range(4):
                    nc.tensor.matmul(ps_mu[:], ones_full[:],
                                     wt_view[:, j, r * RN:(r + 1) * RN],
                                     start=(j == 0), stop=(j == 3),
                                     skip_group_check=True)
                    nc.tensor.matmul(ps_var[:], ones_full[:], wsq[:, j, :],
                                     start=(j == 0), stop=(j == 3),
                                     skip_group_check=True)
                mu = sb2.tile([128, RN], F32, tag="mu_sb", bufs=1, name=f"mu_{r}")
                nc.scalar.mul(mu[:], ps_mu[:], 1.0 / D)
                ex2 = sb2.tile([128, RN], F32, tag="ex2_sb", bufs=1, name=f"ex2_{r}")
                nc.scalar.mul(ex2[:], ps_var[:], 1.0 / D)
                var = sb2.tile([128, RN], F32, tag="var_sb", bufs=1, name=f"var_{r}")
                nc.vector.tensor_mul(var[:], mu[:], mu[:])
                nc.vector.tensor_sub(var[:], ex2[:], var[:])
                sd = sb2.tile([128, RN], F32, tag="sd", bufs=1, name=f"sd_{r}")
                nc.scalar.activation(sd[:], var[:], AF.Sqrt, bias=eps_sb[:])
                rstd = sb2.tile([128, RN], F32, tag="rstd", bufs=1, name=f"rstd_{r}")
                nc.vector.reciprocal(rstd[:], sd[:])
                for j in range(4):
                    tmp = sb2.tile([128, RN], F32, tag="lntmp", name=f"lnt_{r}_{j}")
                    nc.vector.tensor_sub(tmp[:], wt_view[:, j, r * RN:(r + 1) * RN],
                                         mu[:])
                    nc.vector.tensor_mul(tmp[:], tmp[:], rstd[:])
                    nc.scalar.activation(wtn[:, j, r * RN:(r + 1) * RN], tmp[:],
                                         AF.Gelu,
                                         bias=bias_cols["ln_b_c"][:, j:j + 1],
                                         scale=bias_cols["ln_g_c"][:, j:j + 1])

            # ---- pq/pk projections (bf16) ----
            pqT = p2.tile([128, 4, WN], BF16, tag="pqT")
            pkT = p2.tile([128, 4, WN], BF16, tag="pkT")
            for dst, wname, bname in ((pqT, "wpq", "bpq_c"), (pkT, "wpk", "bpk_c")):
                for j in range(4):
                    ps = ps2.tile([128, WN], F32, tag="ps2", name=f"pp_{wname}_{j}")
                    for dk in range(4):
                        nc.tensor.matmul(ps[:], W[wname][:, dk, j * 128:(j + 1) * 128],
                                         wtn[:, dk, :], start=(dk == 0), stop=(dk == 3))
                    nc.scalar.activation(dst[:, j, :], ps[:], AF.Identity,
                                         bias=bias_cols[bname][:, j:j + 1], scale=1.0)

            # ---- PSA: es fp8, den via DoubleRow vs const 1/16, attn8 fp8 ----
            attn8 = p2.tile([128, H, 2, 2, WN], FP8, tag="attn8")
            zt = p2.tile([128, 4, QLEN], BF16, tag="zt")

            def psa_scores(h):
                base = (h % 2) * 64
                es = sb2.tile([128, 4, WN], FP8, tag="psa_exp", bufs=3,
                              name=f"es_{h}")
                for cc in range(4):
                    ps = ps2.tile([128, WN], F32, tag="ps2", name=f"st_{h}_{cc}")
                    nc.tensor.matmul(
                        ps[:], pkT[base:base + 64, h // 2, cc * 128:(cc + 1) * 128],
                        pqT[base:base + 64, h // 2, :], start=True, stop=True)
                    nc.scalar.activation(es[:, cc, :], ps[:], AF.Exp, scale=SCALE)
                return es

            def psa_norm(h, es):
                ps_den = ps2b.tile([128, WN], F32, tag="psa_den", name=f"d_{h}")
                for g in range(2):
                    nc.tensor.matmul(ps_den[:], inv16_full[:],
                                     es[:, 2 * g:2 * g + 2, :],
                                     start=(g == 0), stop=(g == 1),
                                     perf_mode=PM.DoubleRow,
                                     skip_group_check=True)
                recipd = sb2.tile([128, WN], F32, tag="psa_recip", name=f"r_{h}")
                nc.vector.reciprocal(recipd[:], ps_den[:])   # = ASC / sum(es)
                for g in range(2):
                    for jj in range(2):
                        nc.vector.tensor_mul(attn8[:, h, g, jj, :],
                                             es[:, 2 * g + jj, :], recipd[:])

            def pout_pair(u):
                """pout for head pair (2u, 2u+1) per slot: DoubleRow matmuls
                must start at PSUM partition 0, so the two heads go to two
                free-dim halves of one [64, 2, WN] psum tile; the Z = pout/16
                + attn_x adds are 64-partition ops split across DVE/gpsimd."""
                for i in range(WIN):
                    po = ps2.tile([64, 2, WN], F32, tag="ps2", name=f"po_{u}_{i}")
                    for half in range(2):
                        h = 2 * u + half
                        for g in range(2):
                            nc.tensor.matmul(
                                po[:, half, :],
                                pv[:, 2 * g:2 * g + 2, i, h * 64:(h + 1) * 64],
                                attn8[:, h, g, :, :],
                                start=(g == 0), stop=(g == 1),
                                perf_mode=PM.DoubleRow,
                                skip_group_check=True)
                    for half in range(2):
                        hb = half * 64
                        zv = zt[hb:hb + 64, u, :].rearrange("p (w i) -> p w i", i=WIN)
                        av = axt[hb:hb + 64, u, :].rearrange("p (w s) -> p w s", s=PW)
                        # gpsimd cannot read PSUM; these all live on DVE
                        nc.vector.scalar_tensor_tensor(
                            zv[:, :, i], po[:, half, :],
                            1.0 / ASC, av[:, :, 1 + i], ALU.mult, ALU.add)

            es_prev = psa_scores(0)
            for h in range(1, H):
                es_h = psa_scores(h)
                psa_norm(h - 1, es_prev)
                es_prev = es_h
            psa_norm(H - 1, es_prev)
            for u in range(4):
                pout_pair(u)

            # ---- final projection (paired output stores) ----
            outv = out.ap().rearrange("(g tt p) d -> g p tt d", tt=2, p=128)
            for g in range(QLEN // 256):
                o_sb = sb2.tile([128, 2, D], BF16, tag="osb", bufs=3, name=f"osb_{g}")
                for q in range(2):
                    tt = g * 2 + q
                    ps = ps2.tile([128, D], F32, tag="ps2", name=f"fin_{tt}")
                    for dk in range(4):
                        nc.tensor.matmul(ps[:], zt[:, dk, tt * 128:(tt + 1) * 128],
                                         W["wo"][:, dk, :], start=(dk == 0), stop=False,
                                         skip_group_check=True)
                    nc.tensor.matmul(ps[:], ones_row[:], bo_sb[:], start=False,
                                     stop=True, skip_group_check=True)
                    nc.scalar.copy(o_sb[:, q, :], ps[:])
                nc.sync.dma_start(out=outv[g], in_=o_sb[:])


_NC_CACHE = None


def _get_program():
    global _NC_CACHE
    if _NC_CACHE is None:
        _NC_CACHE = build_program()
    return _NC_CACHE


def _host_consts(Wk, bk, Wv, bv, Wq, bq, ln_g, ln_b, Wpq, bpq, Wpk, bpk, Wo, bo):
    bf = ml_dtypes.bfloat16
    f8 = ml_dtypes.float8_e4m3
    col = lambda b: np.asarray(b, np.float32).reshape(4, 128).T.copy()

    def pack8(Wm):
        w = (np.asarray(Wm, np.float32) * WSC).astype(f8)
        # [f_in, of] -> [p, g, j, of] with f_in = g*256 + 2p + j
        return np.ascontiguousarray(w.reshape(2, 128, 2, D).transpose(1, 0, 2, 3))

    consts = {
        "wq8": pack8(Wq), "wk8": pack8(Wk),
        "wv": np.asarray(Wv, np.float32).astype(bf),
        "wpq": np.asarray(Wpq, np.float32).astype(bf),
        "wpk": np.asarray(Wpk, np.float32).astype(bf),
        "wo": np.asarray(Wo, np.float32).astype(bf),
        "bq_c": col(bq), "bk_c": col(bk),
        "bpq_c": col(bpq), "bpk_c": col(bpk),
        "ln_g_c": col(ln_g), "ln_b_c": col(ln_b),
        "bv_r": np.asarray(bv, np.float32).reshape(1, D).astype(bf),
        "bo_r": np.asarray(bo, np.float32).reshape(1, D).astype(bf),
    }
    m = np.zeros((128, 128), np.float32)
    for g in range(16):
        m[g * PW:(g + 1) * PW, g * PW:(g + 1) * PW] = 1.0
    consts["bmask"] = m.astype(bf)
    return consts


def kernel(k, v, q, query_len, Wk, bk, Wv, bv, Wq, bq, ln_g, ln_b,
           Wpq, bpq, Wpk, bpk, Wo, bo):
    nc = _get_program()
    consts = _host_consts(Wk, bk, Wv, bv, Wq, bq, ln_g, ln_b,
                          Wpq, bpq, Wpk, bpk, Wo, bo)
    k = np.asarray(k, np.float32)
    v = np.asarray(v, np.float32)
    q = np.asarray(q, np.float32)
    in_maps = []
    for b in range(B):
        m = {"q": np.ascontiguousarray(q[b]), "k": np.ascontiguousarray(k[b]),
             "v": np.ascontiguousarray(v[b])}
        m.update(consts)
        in_maps.append(m)
    res = run_bass_kernel_spmd(nc, in_maps, core_ids=list(range(B)))
    return np.stack([res.results[b]["out"].astype(np.float32) for b in range(B)],
                    axis=0)


if __name__ == "__main__":
    nc = build_program()
    print("program built ok")
